# revision 11
# baseline (speedup 1.0000x reference)
"""Trainium2 Bass kernel for nn_DeepBSpline (per-channel uniform-knot linear
B-spline activation with linear extrapolation).

Approach: the whole op (clamp + bin + two gathers + lerp + extrapolation) is,
per channel, a single continuous piecewise-linear function of x with at most
50 pieces whose kinks sit at the (compile-time-known) knot grid.  At call
time the host compresses the coefficient table into its minimal relu basis

    f_c(x) = alpha_c + beta_c * x + sum_j D_cj * relu(x - b_cj)

keeping only kinks with a non-negligible slope change.  The device kernel is
then gather-free: one fused multiply-add (VectorE tensor_scalar) plus, per
kept kink, one biased Relu (ScalarE activation) and one fused multiply-add
(VectorE scalar_tensor_tensor), with per-partition scalars carrying the
per-channel constants.  The number of kept kinks T is the only thing baked
into the program; all values flow through a small "consts" input tensor, so
the compiled NEFF is reusable across coefficient values with the same T.

The op is HBM-bandwidth bound (pure streaming elementwise), so device I/O is
done in bfloat16: the host casts x to bf16, the device computes in bf16
(per-partition scalar constants stay fp32), and the bf16 result is upcast on
the host.  This halves HBM traffic vs fp32.  bf16 adds ~2^-9 relative
rounding per step, orders of magnitude below the 2e-2 gate (the function has
slope <= max per-bin slope, so x-quantization error stays relative).

Sharding: data-parallel over the batch dim — 8 cores x 2 batches each; each
core's (2, 64, 256, 256) slab is viewed as [128 partitions, 65536] with
partition p = b*64 + c, so per-channel constants become per-partition scalars.
"""

import os
import sys

import numpy as np

for _p in ("/opt/trn_rl_repo", "/root/.axon_site", "/root/.axon_site/_ro/trn_rl_repo",
           "/root/.axon_site/_ro/pypackages"):
    if os.path.isdir(_p) and _p not in sys.path:
        sys.path.append(_p)

import ml_dtypes

GRID = 0.16
SIZE = 51
HALF = SIZE // 2
C = 64
N_BATCH = 16
HW = 256 * 256
N_CORES = 8
P = 128                      # partitions = 2 batches x 64 channels
BATCH_PER_CORE = N_BATCH // N_CORES
FREE = BATCH_PER_CORE * C * HW // P   # 65536 free-dim elements per partition
F_TILE = 8192
IO_NP = ml_dtypes.bfloat16


def _build_pwl(coefficients_vect, tol_rel=1e-4):
    """Compress the spline table to relu-basis PWL coefficients (float64).

    Returns alpha[C], beta[C], terms (per channel list of (kink_x, slope_delta)),
    and the max term count across channels.
    """
    cv = np.asarray(coefficients_vect, np.float64).reshape(C, SIZE)
    slopes_x = np.diff(cv, axis=1) / GRID          # (C, 50) per-bin slopes
    dd = np.diff(slopes_x, axis=1)                 # (C, 49) slope changes at knots 1..49
    scale = np.abs(dd).max() + 1e-30
    keep = np.abs(dd) > tol_rel * scale
    alpha = np.empty(C)
    beta = np.empty(C)
    terms = []
    max_terms = 0
    for c in range(C):
        ks = [0] + list(np.nonzero(keep[c])[0] + 1) + [SIZE - 1]
        # refit chords so the PWL interpolates the exact table values at the
        # kept kinks and both endpoints
        k0, k1 = ks[0], ks[1]
        s0 = (cv[c, k1] - cv[c, k0]) / ((k1 - k0) * GRID)
        beta[c] = s0
        alpha[c] = cv[c, k0] - (k0 - HALF) * GRID * s0
        t = []
        prev_s = s0
        for i in range(1, len(ks) - 1):
            ka, kb = ks[i], ks[i + 1]
            s = (cv[c, kb] - cv[c, ka]) / ((kb - ka) * GRID)
            t.append(((ka - HALF) * GRID, s - prev_s))
            prev_s = s
        terms.append(t)
        max_terms = max(max_terms, len(t))
    return alpha, beta, terms, max_terms


def _consts_array(alpha, beta, terms, T):
    """[P, 2+2T] float32: per partition (b*64+c): alpha, beta, (-b_j, D_j)*T."""
    K = 2 + 2 * T
    a = np.zeros((C, K), np.float32)
    a[:, 0] = np.asarray(alpha, np.float32)
    a[:, 1] = np.asarray(beta, np.float32)
    for c in range(C):
        for j, (b, d) in enumerate(terms[c]):
            a[c, 2 + 2 * j] = np.float32(-b)
            a[c, 3 + 2 * j] = np.float32(d)
    return np.tile(a, (P // C, 1)).astype(np.float32)


def _consts_prelu(alpha, beta, terms):
    """[P, 4] float32 per partition: -b, beta1/beta2, beta2, f(b).

    For T==1 the PWL is two lines meeting at b with slopes beta1 = beta and
    beta2 = beta + D, so  f(x) = beta2 * prelu(x - b, beta1/beta2) + f(b).
    """
    a = np.zeros((C, 4), np.float32)
    for c in range(C):
        b, Dv = terms[c][0] if terms[c] else (0.0, 0.0)
        b1 = beta[c]
        b2 = beta[c] + Dv
        a[c, 0] = -b
        a[c, 1] = b1 / b2
        a[c, 2] = b2
        a[c, 3] = alpha[c] + beta[c] * b
    return np.tile(a, (P // C, 1)).astype(np.float32)


def _prelu_ok(beta, terms, T):
    """Prelu path needs T==1 and a non-degenerate right slope everywhere."""
    if T != 1:
        return False
    beta1 = np.asarray(beta)
    beta2 = np.array([beta[c] + (terms[c][0][1] if terms[c] else 0.0)
                      for c in range(C)])
    sc = max(np.abs(beta1).max(), np.abs(beta2).max(), 1e-30)
    return bool(np.abs(beta2).min() > 1e-6 * sc)


def _prelu_fold_ok(alpha, beta, terms, cv):
    """ACT-only path: f = Prelu(beta2*x - beta2*b, beta1/beta2) exactly when
    beta2 > 0 (positive homogeneity) and f(b) ~ 0 (skippable offset)."""
    beta2 = np.array([beta[c] + (terms[c][0][1] if terms[c] else 0.0)
                      for c in range(C)])
    fb = np.array([alpha[c] + beta[c] * (terms[c][0][0] if terms[c] else 0.0)
                   for c in range(C)])
    scale = np.abs(np.asarray(cv)).max() + 1e-30
    return bool(beta2.min() > 0 and np.abs(fb).max() < 1e-5 * scale)


def _consts_prelu_fold(alpha, beta, terms):
    """[P, 3] float32 per partition: -beta2*b (bias), beta2 (scale), beta1/beta2."""
    a = np.zeros((C, 3), np.float32)
    for c in range(C):
        b, Dv = terms[c][0] if terms[c] else (0.0, 0.0)
        b2 = beta[c] + Dv
        a[c, 0] = -b2 * b
        a[c, 1] = b2
        a[c, 2] = beta[c] / b2
    return np.tile(a, (P // C, 1)).astype(np.float32)


# Variable tile schedule: small tiles at both ends so the pipeline fills
# fast (first ACT can start after a 1.3us load instead of 5.2us) and drains
# fine-grained (the DMA engines never sit idle waiting for one big final
# ACT+store); big tiles in the middle keep per-tile overhead amortized.
TILE_SCHED = (2048, 4096, 8192, 8192, 8192, 8192, 8192, 8192, 4096, 4096, 2048)
assert sum(TILE_SCHED) == FREE


def _build_bass_prelu_fold(free=FREE, sched=TILE_SCHED):
    """ACT-only program: per tile a single ScalarE Prelu, nothing else.

    The op is HBM-bound (per-core DMA engine pool ~25 B/ns x 16 needs
    ~10.4 us/tile for load+store vs 7.3 us of ScalarE Prelu), so the whole
    game is keeping the DMA engines saturated: deep load prefetch (xin
    bufs), load issue on the otherwise-idle GpSimd SWDGE queue, store issue
    on the idle SP queue, and no VectorE stage at all.
    """
    from contextlib import ExitStack

    import concourse.bass as bass
    import concourse.tile as tile
    from concourse import bacc, mybir

    nc = bacc.Bacc("TRN2", target_bir_lowering=False, debug=False,
                   num_devices=N_CORES)
    f32 = mybir.dt.float32
    bf16 = mybir.dt.bfloat16
    x_d = nc.dram_tensor("x", [P, free], bf16, kind="ExternalInput")
    c_d = nc.dram_tensor("consts", [P, 3], f32, kind="ExternalInput")
    o_d = nc.dram_tensor("out", [P, free], bf16, kind="ExternalOutput")
    assert sum(sched) == free

    prelu = mybir.ActivationFunctionType.Prelu

    with tile.TileContext(nc) as tc, ExitStack() as ctx:
        cpool = ctx.enter_context(tc.tile_pool(name="cpool", bufs=1))
        ct = cpool.tile([P, 3], f32)
        nc.sync.dma_start(ct[:], c_d.ap())

        xin = ctx.enter_context(tc.tile_pool(name="xin", bufs=6))
        op = ctx.enter_context(tc.tile_pool(name="op", bufs=4))

        # Loads ride the SP HWDGE queue: unlike the GpSimd SWDGE path (whose
        # inter-dma DRAINs serialize issue), SP lets loads front-run to the
        # full xin depth.  Stores issue from the ACT queue directly after
        # each ACTIVATE — the data is ready by program order, so no
        # cross-engine semaphore hop sits before the store.
        off = 0
        for sz in sched:
            sl = bass.DynSlice(off, sz)
            off += sz
            xt = xin.tile([P, sz], bf16)
            nc.sync.dma_start(xt[:], x_d.ap()[:, sl])

            ot = op.tile([P, sz], bf16)
            nc.scalar.activation(ot[:], xt[:], prelu,
                                 bias=ct[:, 0:1], scale=ct[:, 1:2],
                                 alpha=ct[:, 2:3])

            nc.scalar.dma_start(o_d.ap()[:, sl], ot[:])

    nc.compile()
    return nc


def _build_bass_prelu(free=FREE, f_tile=F_TILE):
    """Prelu-path program: per tile one ScalarE Prelu + one VectorE FMA.

    The op is HBM-bound; DMA issue must never queue behind long compute on
    the same sequencer, so loads go out on the (otherwise idle) PE sequencer
    and stores on the SP sequencer.  ScalarE runs only the Prelu, VectorE
    only the (fast-mode) tensor_scalar.
    """
    from contextlib import ExitStack

    import concourse.bass as bass
    import concourse.tile as tile
    from concourse import bacc, mybir

    nc = bacc.Bacc("TRN2", target_bir_lowering=False, debug=False,
                   num_devices=N_CORES)
    f32 = mybir.dt.float32
    bf16 = mybir.dt.bfloat16
    x_d = nc.dram_tensor("x", [P, free], bf16, kind="ExternalInput")
    c_d = nc.dram_tensor("consts", [P, 4], f32, kind="ExternalInput")
    o_d = nc.dram_tensor("out", [P, free], bf16, kind="ExternalOutput")
    n_tiles = free // f_tile
    assert n_tiles * f_tile == free

    mul = mybir.AluOpType.mult
    add = mybir.AluOpType.add
    prelu = mybir.ActivationFunctionType.Prelu

    with tile.TileContext(nc) as tc, ExitStack() as ctx:
        cpool = ctx.enter_context(tc.tile_pool(name="cpool", bufs=1))
        ct = cpool.tile([P, 4], f32)
        nc.sync.dma_start(ct[:], c_d.ap())

        xin = ctx.enter_context(tc.tile_pool(name="xin", bufs=4))
        pp = ctx.enter_context(tc.tile_pool(name="pp", bufs=2))
        op = ctx.enter_context(tc.tile_pool(name="op", bufs=3))

        for i in range(n_tiles):
            xt = xin.tile([P, f_tile], bf16)
            nc.gpsimd.dma_start(xt[:], x_d.ap()[:, bass.ts(i, f_tile)])

            pt = pp.tile([P, f_tile], bf16)
            nc.scalar.activation(pt[:], xt[:], prelu,
                                 bias=ct[:, 0:1], alpha=ct[:, 1:2])
            ot = op.tile([P, f_tile], bf16)
            nc.vector.tensor_scalar(ot[:], pt[:], ct[:, 2:3], ct[:, 3:4],
                                    mul, add)

            nc.sync.dma_start(o_d.ap()[:, bass.ts(i, f_tile)], ot[:])

    nc.compile()
    return nc


def _build_bass(T, free=FREE, f_tile=F_TILE):
    """Emit + compile the Bass/Tile program for term count T."""
    from contextlib import ExitStack

    import concourse.bass as bass
    import concourse.tile as tile
    from concourse import bacc, mybir

    nc = bacc.Bacc("TRN2", target_bir_lowering=False, debug=False,
                   num_devices=N_CORES)
    f32 = mybir.dt.float32
    bf16 = mybir.dt.bfloat16
    x_d = nc.dram_tensor("x", [P, free], bf16, kind="ExternalInput")
    c_d = nc.dram_tensor("consts", [P, 2 + 2 * T], f32, kind="ExternalInput")
    o_d = nc.dram_tensor("out", [P, free], bf16, kind="ExternalOutput")
    n_tiles = free // f_tile
    assert n_tiles * f_tile == free

    mul = mybir.AluOpType.mult
    add = mybir.AluOpType.add
    relu = mybir.ActivationFunctionType.Relu

    with tile.TileContext(nc) as tc, ExitStack() as ctx:
        cpool = ctx.enter_context(tc.tile_pool(name="cpool", bufs=1))
        ct = cpool.tile([P, 2 + 2 * T], f32)
        nc.sync.dma_start(ct[:], c_d.ap())

        xin = ctx.enter_context(tc.tile_pool(name="xin", bufs=3))
        fp = ctx.enter_context(tc.tile_pool(name="fp", bufs=2))
        rp = ctx.enter_context(tc.tile_pool(name="rp", bufs=2))
        op = ctx.enter_context(tc.tile_pool(name="op", bufs=3))

        for i in range(n_tiles):
            xt = xin.tile([P, f_tile], bf16)
            # loads issued from the ACT sequencer (qACT HWDGE queue) so that
            # stores (qSP via nc.sync) never head-of-line-block the loads
            nc.scalar.dma_start(xt[:], x_d.ap()[:, bass.ts(i, f_tile)])

            acc = fp.tile([P, f_tile], bf16)
            nc.vector.tensor_scalar(acc[:], xt[:], ct[:, 1:2], ct[:, 0:1], mul, add)

            for j in range(T):
                rt = rp.tile([P, f_tile], bf16)
                nc.scalar.activation(rt[:], xt[:], relu,
                                     bias=ct[:, 2 + 2 * j:3 + 2 * j])
                ot = op.tile([P, f_tile], bf16)
                nc.vector.scalar_tensor_tensor(ot[:], rt[:],
                                               ct[:, 3 + 2 * j:4 + 2 * j],
                                               acc[:], mul, add)
                acc = ot

            nc.sync.dma_start(o_d.ap()[:, bass.ts(i, f_tile)], acc[:])

    nc.compile()
    return nc


_NC_CACHE = {}


def _get_nc(T):
    if T not in _NC_CACHE:
        if T == "prelu_fold":
            _NC_CACHE[T] = _build_bass_prelu_fold()
        elif T == "prelu":
            _NC_CACHE[T] = _build_bass_prelu()
        else:
            _NC_CACHE[T] = _build_bass(T)
    return _NC_CACHE[T]


def _prepare(x, coefficients_vect):
    """Compile (or fetch) the program and build per-core input maps."""
    x = np.asarray(x)
    assert x.shape == (N_BATCH, C, 256, 256)
    cv = np.asarray(coefficients_vect, np.float32)

    alpha, beta, terms, T = _build_pwl(cv)
    T = max(T, 1)
    if _prelu_ok(beta, terms, T) and _prelu_fold_ok(alpha, beta, terms, cv):
        consts = _consts_prelu_fold(alpha, beta, terms)
        nc = _get_nc("prelu_fold")
    elif _prelu_ok(beta, terms, T):
        consts = _consts_prelu(alpha, beta, terms)
        nc = _get_nc("prelu")
    else:
        consts = _consts_array(alpha, beta, terms, T)
        nc = _get_nc(T)
    in_maps = [
        {"x": np.ascontiguousarray(
            x[i * BATCH_PER_CORE:(i + 1) * BATCH_PER_CORE]
        ).reshape(P, FREE).astype(IO_NP),
         "consts": consts}
        for i in range(N_CORES)
    ]
    return nc, in_maps


def kernel(x, coefficients_vect, size):
    assert int(size) == SIZE

    from concourse.bass_utils import run_bass_kernel_spmd

    nc, in_maps = _prepare(x, coefficients_vect)
    res = run_bass_kernel_spmd(nc, in_maps, list(range(N_CORES))).results
    out = np.concatenate(
        [r["out"].astype(np.float32).reshape(BATCH_PER_CORE, C, 256, 256)
         for r in res], axis=0
    )
    return out


# revision 12
# speedup vs baseline: 1.0108x; 1.0108x over previous
"""Trainium2 Bass kernel for nn_DeepBSpline (per-channel uniform-knot linear
B-spline activation with linear extrapolation).

Approach: the whole op (clamp + bin + two gathers + lerp + extrapolation) is,
per channel, a single continuous piecewise-linear function of x with at most
50 pieces whose kinks sit at the (compile-time-known) knot grid.  At call
time the host compresses the coefficient table into its minimal relu basis

    f_c(x) = alpha_c + beta_c * x + sum_j D_cj * relu(x - b_cj)

keeping only kinks with a non-negligible slope change.  The device kernel is
then gather-free: one fused multiply-add (VectorE tensor_scalar) plus, per
kept kink, one biased Relu (ScalarE activation) and one fused multiply-add
(VectorE scalar_tensor_tensor), with per-partition scalars carrying the
per-channel constants.  The number of kept kinks T is the only thing baked
into the program; all values flow through a small "consts" input tensor, so
the compiled NEFF is reusable across coefficient values with the same T.

The op is HBM-bandwidth bound (pure streaming elementwise), so device I/O is
done in bfloat16: the host casts x to bf16, the device computes in bf16
(per-partition scalar constants stay fp32), and the bf16 result is upcast on
the host.  This halves HBM traffic vs fp32.  bf16 adds ~2^-9 relative
rounding per step, orders of magnitude below the 2e-2 gate (the function has
slope <= max per-bin slope, so x-quantization error stays relative).

Sharding: data-parallel over the batch dim — 8 cores x 2 batches each; each
core's (2, 64, 256, 256) slab is viewed as [128 partitions, 65536] with
partition p = b*64 + c, so per-channel constants become per-partition scalars.
"""

import os
import sys

import numpy as np

for _p in ("/opt/trn_rl_repo", "/root/.axon_site", "/root/.axon_site/_ro/trn_rl_repo",
           "/root/.axon_site/_ro/pypackages"):
    if os.path.isdir(_p) and _p not in sys.path:
        sys.path.append(_p)

import ml_dtypes

GRID = 0.16
SIZE = 51
HALF = SIZE // 2
C = 64
N_BATCH = 16
HW = 256 * 256
N_CORES = 8
P = 128                      # partitions = 2 batches x 64 channels
BATCH_PER_CORE = N_BATCH // N_CORES
FREE = BATCH_PER_CORE * C * HW // P   # 65536 free-dim elements per partition
F_TILE = 8192
IO_NP = ml_dtypes.bfloat16


def _build_pwl(coefficients_vect, tol_rel=1e-4):
    """Compress the spline table to relu-basis PWL coefficients (float64).

    Returns alpha[C], beta[C], terms (per channel list of (kink_x, slope_delta)),
    and the max term count across channels.
    """
    cv = np.asarray(coefficients_vect, np.float64).reshape(C, SIZE)
    slopes_x = np.diff(cv, axis=1) / GRID          # (C, 50) per-bin slopes
    dd = np.diff(slopes_x, axis=1)                 # (C, 49) slope changes at knots 1..49
    scale = np.abs(dd).max() + 1e-30
    keep = np.abs(dd) > tol_rel * scale
    alpha = np.empty(C)
    beta = np.empty(C)
    terms = []
    max_terms = 0
    for c in range(C):
        ks = [0] + list(np.nonzero(keep[c])[0] + 1) + [SIZE - 1]
        # refit chords so the PWL interpolates the exact table values at the
        # kept kinks and both endpoints
        k0, k1 = ks[0], ks[1]
        s0 = (cv[c, k1] - cv[c, k0]) / ((k1 - k0) * GRID)
        beta[c] = s0
        alpha[c] = cv[c, k0] - (k0 - HALF) * GRID * s0
        t = []
        prev_s = s0
        for i in range(1, len(ks) - 1):
            ka, kb = ks[i], ks[i + 1]
            s = (cv[c, kb] - cv[c, ka]) / ((kb - ka) * GRID)
            t.append(((ka - HALF) * GRID, s - prev_s))
            prev_s = s
        terms.append(t)
        max_terms = max(max_terms, len(t))
    return alpha, beta, terms, max_terms


def _consts_array(alpha, beta, terms, T):
    """[P, 2+2T] float32: per partition (b*64+c): alpha, beta, (-b_j, D_j)*T."""
    K = 2 + 2 * T
    a = np.zeros((C, K), np.float32)
    a[:, 0] = np.asarray(alpha, np.float32)
    a[:, 1] = np.asarray(beta, np.float32)
    for c in range(C):
        for j, (b, d) in enumerate(terms[c]):
            a[c, 2 + 2 * j] = np.float32(-b)
            a[c, 3 + 2 * j] = np.float32(d)
    return np.tile(a, (P // C, 1)).astype(np.float32)


def _consts_prelu(alpha, beta, terms):
    """[P, 4] float32 per partition: -b, beta1/beta2, beta2, f(b).

    For T==1 the PWL is two lines meeting at b with slopes beta1 = beta and
    beta2 = beta + D, so  f(x) = beta2 * prelu(x - b, beta1/beta2) + f(b).
    """
    a = np.zeros((C, 4), np.float32)
    for c in range(C):
        b, Dv = terms[c][0] if terms[c] else (0.0, 0.0)
        b1 = beta[c]
        b2 = beta[c] + Dv
        a[c, 0] = -b
        a[c, 1] = b1 / b2
        a[c, 2] = b2
        a[c, 3] = alpha[c] + beta[c] * b
    return np.tile(a, (P // C, 1)).astype(np.float32)


def _prelu_ok(beta, terms, T):
    """Prelu path needs T==1 and a non-degenerate right slope everywhere."""
    if T != 1:
        return False
    beta1 = np.asarray(beta)
    beta2 = np.array([beta[c] + (terms[c][0][1] if terms[c] else 0.0)
                      for c in range(C)])
    sc = max(np.abs(beta1).max(), np.abs(beta2).max(), 1e-30)
    return bool(np.abs(beta2).min() > 1e-6 * sc)


def _prelu_fold_ok(alpha, beta, terms, cv):
    """ACT-only path: f = Prelu(beta2*x - beta2*b, beta1/beta2) exactly when
    beta2 > 0 (positive homogeneity) and f(b) ~ 0 (skippable offset)."""
    beta2 = np.array([beta[c] + (terms[c][0][1] if terms[c] else 0.0)
                      for c in range(C)])
    fb = np.array([alpha[c] + beta[c] * (terms[c][0][0] if terms[c] else 0.0)
                   for c in range(C)])
    scale = np.abs(np.asarray(cv)).max() + 1e-30
    return bool(beta2.min() > 0 and np.abs(fb).max() < 1e-5 * scale)


def _consts_prelu_fold(alpha, beta, terms):
    """[P, 3] float32 per partition: -beta2*b (bias), beta2 (scale), beta1/beta2."""
    a = np.zeros((C, 3), np.float32)
    for c in range(C):
        b, Dv = terms[c][0] if terms[c] else (0.0, 0.0)
        b2 = beta[c] + Dv
        a[c, 0] = -b2 * b
        a[c, 1] = b2
        a[c, 2] = beta[c] / b2
    return np.tile(a, (P // C, 1)).astype(np.float32)


# Variable tile schedule: small tiles at both ends so the pipeline fills
# fast (first ACT can start after a 1.3us load instead of 5.2us) and drains
# fine-grained (the DMA engines never sit idle waiting for one big final
# ACT+store); big tiles in the middle keep per-tile overhead amortized.
TILE_SCHED = (2048, 4096, 8192, 8192, 8192, 8192, 8192, 8192, 4096, 4096, 2048)
assert sum(TILE_SCHED) == FREE


def _build_bass_prelu_fold(free=FREE, sched=TILE_SCHED):
    """ACT-only program: per tile a single ScalarE Prelu, nothing else.

    The op is HBM-bound (per-core DMA engine pool ~25 B/ns x 16 needs
    ~10.4 us/tile for load+store vs 7.3 us of ScalarE Prelu), so the whole
    game is keeping the DMA engines saturated: deep load prefetch (xin
    bufs), load issue on the otherwise-idle GpSimd SWDGE queue, store issue
    on the idle SP queue, and no VectorE stage at all.
    """
    from contextlib import ExitStack

    import concourse.bass as bass
    import concourse.tile as tile
    from concourse import bacc, mybir

    nc = bacc.Bacc("TRN2", target_bir_lowering=False, debug=False,
                   num_devices=N_CORES)
    f32 = mybir.dt.float32
    bf16 = mybir.dt.bfloat16
    x_d = nc.dram_tensor("x", [P, free], bf16, kind="ExternalInput")
    c_d = nc.dram_tensor("consts", [P, 3], f32, kind="ExternalInput")
    o_d = nc.dram_tensor("out", [P, free], bf16, kind="ExternalOutput")
    assert sum(sched) == free

    prelu = mybir.ActivationFunctionType.Prelu

    with tile.TileContext(nc) as tc, ExitStack() as ctx:
        cpool = ctx.enter_context(tc.tile_pool(name="cpool", bufs=1))
        ct = cpool.tile([P, 3], f32)
        nc.gpsimd.dma_start(ct[:], c_d.ap())

        xin = ctx.enter_context(tc.tile_pool(name="xin", bufs=6))
        op = ctx.enter_context(tc.tile_pool(name="op", bufs=4))

        # Loads ride the SP HWDGE queue so they pipeline to the full xin
        # depth (SWDGE flushes between dma_starts, serializing a load
        # stream).  Stores go out on the GpSimd SWDGE queue: they only
        # become ready at ACT rate (~7.3us/tile), so SWDGE's flush-per-dma
        # never throttles them, and keeping them off the load queue avoids
        # head-of-line blocking.
        off = 0
        for sz in sched:
            sl = bass.DynSlice(off, sz)
            off += sz
            xt = xin.tile([P, sz], bf16)
            nc.sync.dma_start(xt[:], x_d.ap()[:, sl])

            ot = op.tile([P, sz], bf16)
            nc.scalar.activation(ot[:], xt[:], prelu,
                                 bias=ct[:, 0:1], scale=ct[:, 1:2],
                                 alpha=ct[:, 2:3])

            nc.gpsimd.dma_start(o_d.ap()[:, sl], ot[:])

    nc.compile()
    return nc


def _build_bass_prelu(free=FREE, f_tile=F_TILE):
    """Prelu-path program: per tile one ScalarE Prelu + one VectorE FMA.

    The op is HBM-bound; DMA issue must never queue behind long compute on
    the same sequencer, so loads go out on the (otherwise idle) PE sequencer
    and stores on the SP sequencer.  ScalarE runs only the Prelu, VectorE
    only the (fast-mode) tensor_scalar.
    """
    from contextlib import ExitStack

    import concourse.bass as bass
    import concourse.tile as tile
    from concourse import bacc, mybir

    nc = bacc.Bacc("TRN2", target_bir_lowering=False, debug=False,
                   num_devices=N_CORES)
    f32 = mybir.dt.float32
    bf16 = mybir.dt.bfloat16
    x_d = nc.dram_tensor("x", [P, free], bf16, kind="ExternalInput")
    c_d = nc.dram_tensor("consts", [P, 4], f32, kind="ExternalInput")
    o_d = nc.dram_tensor("out", [P, free], bf16, kind="ExternalOutput")
    n_tiles = free // f_tile
    assert n_tiles * f_tile == free

    mul = mybir.AluOpType.mult
    add = mybir.AluOpType.add
    prelu = mybir.ActivationFunctionType.Prelu

    with tile.TileContext(nc) as tc, ExitStack() as ctx:
        cpool = ctx.enter_context(tc.tile_pool(name="cpool", bufs=1))
        ct = cpool.tile([P, 4], f32)
        nc.sync.dma_start(ct[:], c_d.ap())

        xin = ctx.enter_context(tc.tile_pool(name="xin", bufs=4))
        pp = ctx.enter_context(tc.tile_pool(name="pp", bufs=2))
        op = ctx.enter_context(tc.tile_pool(name="op", bufs=3))

        for i in range(n_tiles):
            xt = xin.tile([P, f_tile], bf16)
            nc.gpsimd.dma_start(xt[:], x_d.ap()[:, bass.ts(i, f_tile)])

            pt = pp.tile([P, f_tile], bf16)
            nc.scalar.activation(pt[:], xt[:], prelu,
                                 bias=ct[:, 0:1], alpha=ct[:, 1:2])
            ot = op.tile([P, f_tile], bf16)
            nc.vector.tensor_scalar(ot[:], pt[:], ct[:, 2:3], ct[:, 3:4],
                                    mul, add)

            nc.sync.dma_start(o_d.ap()[:, bass.ts(i, f_tile)], ot[:])

    nc.compile()
    return nc


def _build_bass(T, free=FREE, f_tile=F_TILE):
    """Emit + compile the Bass/Tile program for term count T."""
    from contextlib import ExitStack

    import concourse.bass as bass
    import concourse.tile as tile
    from concourse import bacc, mybir

    nc = bacc.Bacc("TRN2", target_bir_lowering=False, debug=False,
                   num_devices=N_CORES)
    f32 = mybir.dt.float32
    bf16 = mybir.dt.bfloat16
    x_d = nc.dram_tensor("x", [P, free], bf16, kind="ExternalInput")
    c_d = nc.dram_tensor("consts", [P, 2 + 2 * T], f32, kind="ExternalInput")
    o_d = nc.dram_tensor("out", [P, free], bf16, kind="ExternalOutput")
    n_tiles = free // f_tile
    assert n_tiles * f_tile == free

    mul = mybir.AluOpType.mult
    add = mybir.AluOpType.add
    relu = mybir.ActivationFunctionType.Relu

    with tile.TileContext(nc) as tc, ExitStack() as ctx:
        cpool = ctx.enter_context(tc.tile_pool(name="cpool", bufs=1))
        ct = cpool.tile([P, 2 + 2 * T], f32)
        nc.sync.dma_start(ct[:], c_d.ap())

        xin = ctx.enter_context(tc.tile_pool(name="xin", bufs=3))
        fp = ctx.enter_context(tc.tile_pool(name="fp", bufs=2))
        rp = ctx.enter_context(tc.tile_pool(name="rp", bufs=2))
        op = ctx.enter_context(tc.tile_pool(name="op", bufs=3))

        for i in range(n_tiles):
            xt = xin.tile([P, f_tile], bf16)
            # loads issued from the ACT sequencer (qACT HWDGE queue) so that
            # stores (qSP via nc.sync) never head-of-line-block the loads
            nc.scalar.dma_start(xt[:], x_d.ap()[:, bass.ts(i, f_tile)])

            acc = fp.tile([P, f_tile], bf16)
            nc.vector.tensor_scalar(acc[:], xt[:], ct[:, 1:2], ct[:, 0:1], mul, add)

            for j in range(T):
                rt = rp.tile([P, f_tile], bf16)
                nc.scalar.activation(rt[:], xt[:], relu,
                                     bias=ct[:, 2 + 2 * j:3 + 2 * j])
                ot = op.tile([P, f_tile], bf16)
                nc.vector.scalar_tensor_tensor(ot[:], rt[:],
                                               ct[:, 3 + 2 * j:4 + 2 * j],
                                               acc[:], mul, add)
                acc = ot

            nc.sync.dma_start(o_d.ap()[:, bass.ts(i, f_tile)], acc[:])

    nc.compile()
    return nc


_NC_CACHE = {}


def _get_nc(T):
    if T not in _NC_CACHE:
        if T == "prelu_fold":
            _NC_CACHE[T] = _build_bass_prelu_fold()
        elif T == "prelu":
            _NC_CACHE[T] = _build_bass_prelu()
        else:
            _NC_CACHE[T] = _build_bass(T)
    return _NC_CACHE[T]


def _prepare(x, coefficients_vect):
    """Compile (or fetch) the program and build per-core input maps."""
    x = np.asarray(x)
    assert x.shape == (N_BATCH, C, 256, 256)
    cv = np.asarray(coefficients_vect, np.float32)

    alpha, beta, terms, T = _build_pwl(cv)
    T = max(T, 1)
    if _prelu_ok(beta, terms, T) and _prelu_fold_ok(alpha, beta, terms, cv):
        consts = _consts_prelu_fold(alpha, beta, terms)
        nc = _get_nc("prelu_fold")
    elif _prelu_ok(beta, terms, T):
        consts = _consts_prelu(alpha, beta, terms)
        nc = _get_nc("prelu")
    else:
        consts = _consts_array(alpha, beta, terms, T)
        nc = _get_nc(T)
    in_maps = [
        {"x": np.ascontiguousarray(
            x[i * BATCH_PER_CORE:(i + 1) * BATCH_PER_CORE]
        ).reshape(P, FREE).astype(IO_NP),
         "consts": consts}
        for i in range(N_CORES)
    ]
    return nc, in_maps


def kernel(x, coefficients_vect, size):
    assert int(size) == SIZE

    from concourse.bass_utils import run_bass_kernel_spmd

    nc, in_maps = _prepare(x, coefficients_vect)
    res = run_bass_kernel_spmd(nc, in_maps, list(range(N_CORES))).results
    out = np.concatenate(
        [r["out"].astype(np.float32).reshape(BATCH_PER_CORE, C, 256, 256)
         for r in res], axis=0
    )
    return out


# revision 13
# speedup vs baseline: 1.0178x; 1.0069x over previous
"""Trainium2 Bass kernel for nn_DeepBSpline (per-channel uniform-knot linear
B-spline activation with linear extrapolation).

Approach: the whole op (clamp + bin + two gathers + lerp + extrapolation) is,
per channel, a single continuous piecewise-linear function of x with at most
50 pieces whose kinks sit at the (compile-time-known) knot grid.  At call
time the host compresses the coefficient table into its minimal relu basis

    f_c(x) = alpha_c + beta_c * x + sum_j D_cj * relu(x - b_cj)

keeping only kinks with a non-negligible slope change.  The device kernel is
then gather-free: one fused multiply-add (VectorE tensor_scalar) plus, per
kept kink, one biased Relu (ScalarE activation) and one fused multiply-add
(VectorE scalar_tensor_tensor), with per-partition scalars carrying the
per-channel constants.  The number of kept kinks T is the only thing baked
into the program; all values flow through a small "consts" input tensor, so
the compiled NEFF is reusable across coefficient values with the same T.

The op is HBM-bandwidth bound (pure streaming elementwise), so device I/O is
done in bfloat16: the host casts x to bf16, the device computes in bf16
(per-partition scalar constants stay fp32), and the bf16 result is upcast on
the host.  This halves HBM traffic vs fp32.  bf16 adds ~2^-9 relative
rounding per step, orders of magnitude below the 2e-2 gate (the function has
slope <= max per-bin slope, so x-quantization error stays relative).

Sharding: data-parallel over the batch dim — 8 cores x 2 batches each; each
core's (2, 64, 256, 256) slab is viewed as [128 partitions, 65536] with
partition p = b*64 + c, so per-channel constants become per-partition scalars.
"""

import os
import sys

import numpy as np

for _p in ("/opt/trn_rl_repo", "/root/.axon_site", "/root/.axon_site/_ro/trn_rl_repo",
           "/root/.axon_site/_ro/pypackages"):
    if os.path.isdir(_p) and _p not in sys.path:
        sys.path.append(_p)

import ml_dtypes

GRID = 0.16
SIZE = 51
HALF = SIZE // 2
C = 64
N_BATCH = 16
HW = 256 * 256
N_CORES = 8
P = 128                      # partitions = 2 batches x 64 channels
BATCH_PER_CORE = N_BATCH // N_CORES
FREE = BATCH_PER_CORE * C * HW // P   # 65536 free-dim elements per partition
F_TILE = 8192
IO_NP = ml_dtypes.bfloat16


def _build_pwl(coefficients_vect, tol_rel=1e-4):
    """Compress the spline table to relu-basis PWL coefficients (float64).

    Returns alpha[C], beta[C], terms (per channel list of (kink_x, slope_delta)),
    and the max term count across channels.
    """
    cv = np.asarray(coefficients_vect, np.float64).reshape(C, SIZE)
    slopes_x = np.diff(cv, axis=1) / GRID          # (C, 50) per-bin slopes
    dd = np.diff(slopes_x, axis=1)                 # (C, 49) slope changes at knots 1..49
    scale = np.abs(dd).max() + 1e-30
    keep = np.abs(dd) > tol_rel * scale
    alpha = np.empty(C)
    beta = np.empty(C)
    terms = []
    max_terms = 0
    for c in range(C):
        ks = [0] + list(np.nonzero(keep[c])[0] + 1) + [SIZE - 1]
        # refit chords so the PWL interpolates the exact table values at the
        # kept kinks and both endpoints
        k0, k1 = ks[0], ks[1]
        s0 = (cv[c, k1] - cv[c, k0]) / ((k1 - k0) * GRID)
        beta[c] = s0
        alpha[c] = cv[c, k0] - (k0 - HALF) * GRID * s0
        t = []
        prev_s = s0
        for i in range(1, len(ks) - 1):
            ka, kb = ks[i], ks[i + 1]
            s = (cv[c, kb] - cv[c, ka]) / ((kb - ka) * GRID)
            t.append(((ka - HALF) * GRID, s - prev_s))
            prev_s = s
        terms.append(t)
        max_terms = max(max_terms, len(t))
    return alpha, beta, terms, max_terms


def _consts_array(alpha, beta, terms, T):
    """[P, 2+2T] float32: per partition (b*64+c): alpha, beta, (-b_j, D_j)*T."""
    K = 2 + 2 * T
    a = np.zeros((C, K), np.float32)
    a[:, 0] = np.asarray(alpha, np.float32)
    a[:, 1] = np.asarray(beta, np.float32)
    for c in range(C):
        for j, (b, d) in enumerate(terms[c]):
            a[c, 2 + 2 * j] = np.float32(-b)
            a[c, 3 + 2 * j] = np.float32(d)
    return np.tile(a, (P // C, 1)).astype(np.float32)


def _consts_prelu(alpha, beta, terms):
    """[P, 4] float32 per partition: -b, beta1/beta2, beta2, f(b).

    For T==1 the PWL is two lines meeting at b with slopes beta1 = beta and
    beta2 = beta + D, so  f(x) = beta2 * prelu(x - b, beta1/beta2) + f(b).
    """
    a = np.zeros((C, 4), np.float32)
    for c in range(C):
        b, Dv = terms[c][0] if terms[c] else (0.0, 0.0)
        b1 = beta[c]
        b2 = beta[c] + Dv
        a[c, 0] = -b
        a[c, 1] = b1 / b2
        a[c, 2] = b2
        a[c, 3] = alpha[c] + beta[c] * b
    return np.tile(a, (P // C, 1)).astype(np.float32)


def _prelu_ok(beta, terms, T):
    """Prelu path needs T==1 and a non-degenerate right slope everywhere."""
    if T != 1:
        return False
    beta1 = np.asarray(beta)
    beta2 = np.array([beta[c] + (terms[c][0][1] if terms[c] else 0.0)
                      for c in range(C)])
    sc = max(np.abs(beta1).max(), np.abs(beta2).max(), 1e-30)
    return bool(np.abs(beta2).min() > 1e-6 * sc)


def _prelu_fold_ok(alpha, beta, terms, cv):
    """ACT-only path: f = Prelu(beta2*x - beta2*b, beta1/beta2) exactly when
    beta2 > 0 (positive homogeneity) and f(b) ~ 0 (skippable offset)."""
    beta2 = np.array([beta[c] + (terms[c][0][1] if terms[c] else 0.0)
                      for c in range(C)])
    fb = np.array([alpha[c] + beta[c] * (terms[c][0][0] if terms[c] else 0.0)
                   for c in range(C)])
    scale = np.abs(np.asarray(cv)).max() + 1e-30
    return bool(beta2.min() > 0 and np.abs(fb).max() < 1e-5 * scale)


def _consts_prelu_fold(alpha, beta, terms):
    """[P, 3] float32 per partition: -beta2*b (bias), beta2 (scale), beta1/beta2."""
    a = np.zeros((C, 3), np.float32)
    for c in range(C):
        b, Dv = terms[c][0] if terms[c] else (0.0, 0.0)
        b2 = beta[c] + Dv
        a[c, 0] = -b2 * b
        a[c, 1] = b2
        a[c, 2] = beta[c] / b2
    return np.tile(a, (P // C, 1)).astype(np.float32)


# Variable tile schedule: small tiles at both ends so the pipeline fills
# fast (first ACT can start after a 1.3us load instead of 5.2us) and drains
# fine-grained (the DMA engines never sit idle waiting for one big final
# ACT+store); big tiles in the middle keep per-tile overhead amortized.
TILE_SCHED = (2048, 4096, 8192, 8192, 8192, 8192, 8192, 8192, 4096, 4096, 2048)
assert sum(TILE_SCHED) == FREE


def _build_bass_prelu_fold(free=FREE, sched=TILE_SCHED):
    """ACT-only program: per tile a single ScalarE Prelu, nothing else.

    The op is HBM-bound (per-core DMA engine pool ~25 B/ns x 16 needs
    ~10.4 us/tile for load+store vs 7.3 us of ScalarE Prelu), so the whole
    game is keeping the DMA engines saturated: deep load prefetch (xin
    bufs), load issue on the otherwise-idle GpSimd SWDGE queue, store issue
    on the idle SP queue, and no VectorE stage at all.
    """
    from contextlib import ExitStack

    import concourse.bass as bass
    import concourse.tile as tile
    from concourse import bacc, mybir

    nc = bacc.Bacc("TRN2", target_bir_lowering=False, debug=False,
                   num_devices=N_CORES)
    f32 = mybir.dt.float32
    bf16 = mybir.dt.bfloat16
    x_d = nc.dram_tensor("x", [P, free], bf16, kind="ExternalInput")
    c_d = nc.dram_tensor("consts", [P, 3], f32, kind="ExternalInput")
    o_d = nc.dram_tensor("out", [P, free], bf16, kind="ExternalOutput")
    assert sum(sched) == free

    prelu = mybir.ActivationFunctionType.Prelu

    with tile.TileContext(nc) as tc, ExitStack() as ctx:
        cpool = ctx.enter_context(tc.tile_pool(name="cpool", bufs=1))
        ct = cpool.tile([P, 3], f32)
        nc.gpsimd.dma_start(ct[:], c_d.ap())

        xin = ctx.enter_context(tc.tile_pool(name="xin", bufs=8))
        op = ctx.enter_context(tc.tile_pool(name="op", bufs=4))

        # Loads ride the SP HWDGE queue so they pipeline to the full xin
        # depth (SWDGE flushes between dma_starts, serializing a load
        # stream).  Stores go out on the GpSimd SWDGE queue: they only
        # become ready at ACT rate (~7.3us/tile), so SWDGE's flush-per-dma
        # never throttles them, and keeping them off the load queue avoids
        # head-of-line blocking.
        off = 0
        for sz in sched:
            sl = bass.DynSlice(off, sz)
            off += sz
            xt = xin.tile([P, sz], bf16)
            nc.sync.dma_start(xt[:], x_d.ap()[:, sl])

            ot = op.tile([P, sz], bf16)
            nc.scalar.activation(ot[:], xt[:], prelu,
                                 bias=ct[:, 0:1], scale=ct[:, 1:2],
                                 alpha=ct[:, 2:3])

            nc.gpsimd.dma_start(o_d.ap()[:, sl], ot[:])

    nc.compile()
    return nc


def _build_bass_prelu(free=FREE, f_tile=F_TILE):
    """Prelu-path program: per tile one ScalarE Prelu + one VectorE FMA.

    The op is HBM-bound; DMA issue must never queue behind long compute on
    the same sequencer, so loads go out on the (otherwise idle) PE sequencer
    and stores on the SP sequencer.  ScalarE runs only the Prelu, VectorE
    only the (fast-mode) tensor_scalar.
    """
    from contextlib import ExitStack

    import concourse.bass as bass
    import concourse.tile as tile
    from concourse import bacc, mybir

    nc = bacc.Bacc("TRN2", target_bir_lowering=False, debug=False,
                   num_devices=N_CORES)
    f32 = mybir.dt.float32
    bf16 = mybir.dt.bfloat16
    x_d = nc.dram_tensor("x", [P, free], bf16, kind="ExternalInput")
    c_d = nc.dram_tensor("consts", [P, 4], f32, kind="ExternalInput")
    o_d = nc.dram_tensor("out", [P, free], bf16, kind="ExternalOutput")
    n_tiles = free // f_tile
    assert n_tiles * f_tile == free

    mul = mybir.AluOpType.mult
    add = mybir.AluOpType.add
    prelu = mybir.ActivationFunctionType.Prelu

    with tile.TileContext(nc) as tc, ExitStack() as ctx:
        cpool = ctx.enter_context(tc.tile_pool(name="cpool", bufs=1))
        ct = cpool.tile([P, 4], f32)
        nc.sync.dma_start(ct[:], c_d.ap())

        xin = ctx.enter_context(tc.tile_pool(name="xin", bufs=4))
        pp = ctx.enter_context(tc.tile_pool(name="pp", bufs=2))
        op = ctx.enter_context(tc.tile_pool(name="op", bufs=3))

        for i in range(n_tiles):
            xt = xin.tile([P, f_tile], bf16)
            nc.gpsimd.dma_start(xt[:], x_d.ap()[:, bass.ts(i, f_tile)])

            pt = pp.tile([P, f_tile], bf16)
            nc.scalar.activation(pt[:], xt[:], prelu,
                                 bias=ct[:, 0:1], alpha=ct[:, 1:2])
            ot = op.tile([P, f_tile], bf16)
            nc.vector.tensor_scalar(ot[:], pt[:], ct[:, 2:3], ct[:, 3:4],
                                    mul, add)

            nc.sync.dma_start(o_d.ap()[:, bass.ts(i, f_tile)], ot[:])

    nc.compile()
    return nc


def _build_bass(T, free=FREE, f_tile=F_TILE):
    """Emit + compile the Bass/Tile program for term count T."""
    from contextlib import ExitStack

    import concourse.bass as bass
    import concourse.tile as tile
    from concourse import bacc, mybir

    nc = bacc.Bacc("TRN2", target_bir_lowering=False, debug=False,
                   num_devices=N_CORES)
    f32 = mybir.dt.float32
    bf16 = mybir.dt.bfloat16
    x_d = nc.dram_tensor("x", [P, free], bf16, kind="ExternalInput")
    c_d = nc.dram_tensor("consts", [P, 2 + 2 * T], f32, kind="ExternalInput")
    o_d = nc.dram_tensor("out", [P, free], bf16, kind="ExternalOutput")
    n_tiles = free // f_tile
    assert n_tiles * f_tile == free

    mul = mybir.AluOpType.mult
    add = mybir.AluOpType.add
    relu = mybir.ActivationFunctionType.Relu

    with tile.TileContext(nc) as tc, ExitStack() as ctx:
        cpool = ctx.enter_context(tc.tile_pool(name="cpool", bufs=1))
        ct = cpool.tile([P, 2 + 2 * T], f32)
        nc.sync.dma_start(ct[:], c_d.ap())

        xin = ctx.enter_context(tc.tile_pool(name="xin", bufs=3))
        fp = ctx.enter_context(tc.tile_pool(name="fp", bufs=2))
        rp = ctx.enter_context(tc.tile_pool(name="rp", bufs=2))
        op = ctx.enter_context(tc.tile_pool(name="op", bufs=3))

        for i in range(n_tiles):
            xt = xin.tile([P, f_tile], bf16)
            # loads issued from the ACT sequencer (qACT HWDGE queue) so that
            # stores (qSP via nc.sync) never head-of-line-block the loads
            nc.scalar.dma_start(xt[:], x_d.ap()[:, bass.ts(i, f_tile)])

            acc = fp.tile([P, f_tile], bf16)
            nc.vector.tensor_scalar(acc[:], xt[:], ct[:, 1:2], ct[:, 0:1], mul, add)

            for j in range(T):
                rt = rp.tile([P, f_tile], bf16)
                nc.scalar.activation(rt[:], xt[:], relu,
                                     bias=ct[:, 2 + 2 * j:3 + 2 * j])
                ot = op.tile([P, f_tile], bf16)
                nc.vector.scalar_tensor_tensor(ot[:], rt[:],
                                               ct[:, 3 + 2 * j:4 + 2 * j],
                                               acc[:], mul, add)
                acc = ot

            nc.sync.dma_start(o_d.ap()[:, bass.ts(i, f_tile)], acc[:])

    nc.compile()
    return nc


_NC_CACHE = {}


def _get_nc(T):
    if T not in _NC_CACHE:
        if T == "prelu_fold":
            _NC_CACHE[T] = _build_bass_prelu_fold()
        elif T == "prelu":
            _NC_CACHE[T] = _build_bass_prelu()
        else:
            _NC_CACHE[T] = _build_bass(T)
    return _NC_CACHE[T]


def _prepare(x, coefficients_vect):
    """Compile (or fetch) the program and build per-core input maps."""
    x = np.asarray(x)
    assert x.shape == (N_BATCH, C, 256, 256)
    cv = np.asarray(coefficients_vect, np.float32)

    alpha, beta, terms, T = _build_pwl(cv)
    T = max(T, 1)
    if _prelu_ok(beta, terms, T) and _prelu_fold_ok(alpha, beta, terms, cv):
        consts = _consts_prelu_fold(alpha, beta, terms)
        nc = _get_nc("prelu_fold")
    elif _prelu_ok(beta, terms, T):
        consts = _consts_prelu(alpha, beta, terms)
        nc = _get_nc("prelu")
    else:
        consts = _consts_array(alpha, beta, terms, T)
        nc = _get_nc(T)
    in_maps = [
        {"x": np.ascontiguousarray(
            x[i * BATCH_PER_CORE:(i + 1) * BATCH_PER_CORE]
        ).reshape(P, FREE).astype(IO_NP),
         "consts": consts}
        for i in range(N_CORES)
    ]
    return nc, in_maps


def kernel(x, coefficients_vect, size):
    assert int(size) == SIZE

    from concourse.bass_utils import run_bass_kernel_spmd

    nc, in_maps = _prepare(x, coefficients_vect)
    res = run_bass_kernel_spmd(nc, in_maps, list(range(N_CORES))).results
    out = np.concatenate(
        [r["out"].astype(np.float32).reshape(BATCH_PER_CORE, C, 256, 256)
         for r in res], axis=0
    )
    return out


# revision 15
# speedup vs baseline: 1.1500x; 1.1299x over previous
"""Trainium2 Bass kernel for nn_DeepBSpline (per-channel uniform-knot linear
B-spline activation with linear extrapolation).

Approach: the whole op (clamp + bin + two gathers + lerp + extrapolation) is,
per channel, a single continuous piecewise-linear function of x with at most
50 pieces whose kinks sit at the (compile-time-known) knot grid.  At call
time the host compresses the coefficient table into its minimal relu basis

    f_c(x) = alpha_c + beta_c * x + sum_j D_cj * relu(x - b_cj)

keeping only kinks with a non-negligible slope change.  The device kernel is
then gather-free: one fused multiply-add (VectorE tensor_scalar) plus, per
kept kink, one biased Relu (ScalarE activation) and one fused multiply-add
(VectorE scalar_tensor_tensor), with per-partition scalars carrying the
per-channel constants.  The number of kept kinks T is the only thing baked
into the program; all values flow through a small "consts" input tensor, so
the compiled NEFF is reusable across coefficient values with the same T.

The op is HBM-bandwidth bound (pure streaming elementwise), so device I/O is
done in bfloat16: the host casts x to bf16, the device computes in bf16
(per-partition scalar constants stay fp32), and the bf16 result is upcast on
the host.  This halves HBM traffic vs fp32.  bf16 adds ~2^-9 relative
rounding per step, orders of magnitude below the 2e-2 gate (the function has
slope <= max per-bin slope, so x-quantization error stays relative).

Sharding: data-parallel over the batch dim — 8 cores x 2 batches each; each
core's (2, 64, 256, 256) slab is viewed as [128 partitions, 65536] with
partition p = b*64 + c, so per-channel constants become per-partition scalars.
"""

import os
import sys

import numpy as np

for _p in ("/opt/trn_rl_repo", "/root/.axon_site", "/root/.axon_site/_ro/trn_rl_repo",
           "/root/.axon_site/_ro/pypackages"):
    if os.path.isdir(_p) and _p not in sys.path:
        sys.path.append(_p)

import ml_dtypes

GRID = 0.16
SIZE = 51
HALF = SIZE // 2
C = 64
N_BATCH = 16
HW = 256 * 256
N_CORES = 8
P = 128                      # partitions = 2 batches x 64 channels
BATCH_PER_CORE = N_BATCH // N_CORES
FREE = BATCH_PER_CORE * C * HW // P   # 65536 free-dim elements per partition
F_TILE = 8192
IO_NP = ml_dtypes.bfloat16


def _build_pwl(coefficients_vect, tol_rel=1e-4):
    """Compress the spline table to relu-basis PWL coefficients (float64).

    Returns alpha[C], beta[C], terms (per channel list of (kink_x, slope_delta)),
    and the max term count across channels.
    """
    cv = np.asarray(coefficients_vect, np.float64).reshape(C, SIZE)
    slopes_x = np.diff(cv, axis=1) / GRID          # (C, 50) per-bin slopes
    dd = np.diff(slopes_x, axis=1)                 # (C, 49) slope changes at knots 1..49
    scale = np.abs(dd).max() + 1e-30
    keep = np.abs(dd) > tol_rel * scale
    alpha = np.empty(C)
    beta = np.empty(C)
    terms = []
    max_terms = 0
    for c in range(C):
        ks = [0] + list(np.nonzero(keep[c])[0] + 1) + [SIZE - 1]
        # refit chords so the PWL interpolates the exact table values at the
        # kept kinks and both endpoints
        k0, k1 = ks[0], ks[1]
        s0 = (cv[c, k1] - cv[c, k0]) / ((k1 - k0) * GRID)
        beta[c] = s0
        alpha[c] = cv[c, k0] - (k0 - HALF) * GRID * s0
        t = []
        prev_s = s0
        for i in range(1, len(ks) - 1):
            ka, kb = ks[i], ks[i + 1]
            s = (cv[c, kb] - cv[c, ka]) / ((kb - ka) * GRID)
            t.append(((ka - HALF) * GRID, s - prev_s))
            prev_s = s
        terms.append(t)
        max_terms = max(max_terms, len(t))
    return alpha, beta, terms, max_terms


def _consts_array(alpha, beta, terms, T):
    """[P, 2+2T] float32: per partition (b*64+c): alpha, beta, (-b_j, D_j)*T."""
    K = 2 + 2 * T
    a = np.zeros((C, K), np.float32)
    a[:, 0] = np.asarray(alpha, np.float32)
    a[:, 1] = np.asarray(beta, np.float32)
    for c in range(C):
        for j, (b, d) in enumerate(terms[c]):
            a[c, 2 + 2 * j] = np.float32(-b)
            a[c, 3 + 2 * j] = np.float32(d)
    return np.tile(a, (P // C, 1)).astype(np.float32)


def _consts_prelu(alpha, beta, terms):
    """[P, 4] float32 per partition: -b, beta1/beta2, beta2, f(b).

    For T==1 the PWL is two lines meeting at b with slopes beta1 = beta and
    beta2 = beta + D, so  f(x) = beta2 * prelu(x - b, beta1/beta2) + f(b).
    """
    a = np.zeros((C, 4), np.float32)
    for c in range(C):
        b, Dv = terms[c][0] if terms[c] else (0.0, 0.0)
        b1 = beta[c]
        b2 = beta[c] + Dv
        a[c, 0] = -b
        a[c, 1] = b1 / b2
        a[c, 2] = b2
        a[c, 3] = alpha[c] + beta[c] * b
    return np.tile(a, (P // C, 1)).astype(np.float32)


def _prelu_ok(beta, terms, T):
    """Prelu path needs T==1 and a non-degenerate right slope everywhere."""
    if T != 1:
        return False
    beta1 = np.asarray(beta)
    beta2 = np.array([beta[c] + (terms[c][0][1] if terms[c] else 0.0)
                      for c in range(C)])
    sc = max(np.abs(beta1).max(), np.abs(beta2).max(), 1e-30)
    return bool(np.abs(beta2).min() > 1e-6 * sc)


def _prelu_fold_ok(alpha, beta, terms, cv):
    """ACT-only path: f = Prelu(beta2*x - beta2*b, beta1/beta2) exactly when
    beta2 > 0 (positive homogeneity) and f(b) ~ 0 (skippable offset)."""
    beta2 = np.array([beta[c] + (terms[c][0][1] if terms[c] else 0.0)
                      for c in range(C)])
    fb = np.array([alpha[c] + beta[c] * (terms[c][0][0] if terms[c] else 0.0)
                   for c in range(C)])
    scale = np.abs(np.asarray(cv)).max() + 1e-30
    return bool(beta2.min() > 0 and np.abs(fb).max() < 1e-5 * scale)


def _consts_prelu_fold(alpha, beta, terms):
    """[P, 3] float32 per partition: -beta2*b (bias), beta2 (scale), beta1/beta2."""
    a = np.zeros((C, 3), np.float32)
    for c in range(C):
        b, Dv = terms[c][0] if terms[c] else (0.0, 0.0)
        b2 = beta[c] + Dv
        a[c, 0] = -b2 * b
        a[c, 1] = b2
        a[c, 2] = beta[c] / b2
    return np.tile(a, (P // C, 1)).astype(np.float32)


# Uniform 4096-elem tiles with one xin buffer per tile: the tile-pool
# flow-control guards are embedded as descriptor-level waits in the DMA
# ring, and any guard stall idles the whole 16-engine pool (a pure-DMA
# probe with guarded loads ran 105us vs the ~92us work-conserving bound).
# With bufs == n_tiles the load ring is completely guard-free.
TILE_SCHED = (4096,) * 16
assert sum(TILE_SCHED) == FREE


def _build_bass_prelu_fold(free=FREE, sched=TILE_SCHED):
    """ACT-only program: per tile a single ScalarE Prelu, nothing else.

    The op is HBM-bound (per-core DMA engine pool ~25 B/ns x 16 needs
    ~10.4 us/tile for load+store vs 7.3 us of ScalarE Prelu), so the whole
    game is keeping the DMA engines saturated: deep load prefetch (xin
    bufs), load issue on the otherwise-idle GpSimd SWDGE queue, store issue
    on the idle SP queue, and no VectorE stage at all.
    """
    from contextlib import ExitStack

    import concourse.bass as bass
    import concourse.tile as tile
    from concourse import bacc, mybir

    nc = bacc.Bacc("TRN2", target_bir_lowering=False, debug=False,
                   num_devices=N_CORES)
    f32 = mybir.dt.float32
    bf16 = mybir.dt.bfloat16
    x_d = nc.dram_tensor("x", [P, free], bf16, kind="ExternalInput")
    c_d = nc.dram_tensor("consts", [P, 3], f32, kind="ExternalInput")
    o_d = nc.dram_tensor("out", [P, free], bf16, kind="ExternalOutput")
    assert sum(sched) == free

    prelu = mybir.ActivationFunctionType.Prelu

    with tile.TileContext(nc) as tc, ExitStack() as ctx:
        cpool = ctx.enter_context(tc.tile_pool(name="cpool", bufs=1))
        ct = cpool.tile([P, 3], f32)
        nc.gpsimd.dma_start(ct[:], c_d.ap())

        xin = ctx.enter_context(tc.tile_pool(name="xin", bufs=16))
        op = ctx.enter_context(tc.tile_pool(name="op", bufs=8))

        # Loads ride the SP HWDGE queue so they pipeline to the full xin
        # depth (SWDGE flushes between dma_starts, serializing a load
        # stream).  Stores go out on the GpSimd SWDGE queue: they only
        # become ready at ACT rate (~7.3us/tile), so SWDGE's flush-per-dma
        # never throttles them, and keeping them off the load queue avoids
        # head-of-line blocking.
        off = 0
        for sz in sched:
            sl = bass.DynSlice(off, sz)
            off += sz
            xt = xin.tile([P, sz], bf16)
            nc.sync.dma_start(xt[:], x_d.ap()[:, sl])

            ot = op.tile([P, sz], bf16)
            nc.scalar.activation(ot[:], xt[:], prelu,
                                 bias=ct[:, 0:1], scale=ct[:, 1:2],
                                 alpha=ct[:, 2:3])

            nc.gpsimd.dma_start(o_d.ap()[:, sl], ot[:])

    nc.compile()
    return nc


def _build_bass_prelu(free=FREE, f_tile=F_TILE):
    """Prelu-path program: per tile one ScalarE Prelu + one VectorE FMA.

    The op is HBM-bound; DMA issue must never queue behind long compute on
    the same sequencer, so loads go out on the (otherwise idle) PE sequencer
    and stores on the SP sequencer.  ScalarE runs only the Prelu, VectorE
    only the (fast-mode) tensor_scalar.
    """
    from contextlib import ExitStack

    import concourse.bass as bass
    import concourse.tile as tile
    from concourse import bacc, mybir

    nc = bacc.Bacc("TRN2", target_bir_lowering=False, debug=False,
                   num_devices=N_CORES)
    f32 = mybir.dt.float32
    bf16 = mybir.dt.bfloat16
    x_d = nc.dram_tensor("x", [P, free], bf16, kind="ExternalInput")
    c_d = nc.dram_tensor("consts", [P, 4], f32, kind="ExternalInput")
    o_d = nc.dram_tensor("out", [P, free], bf16, kind="ExternalOutput")
    n_tiles = free // f_tile
    assert n_tiles * f_tile == free

    mul = mybir.AluOpType.mult
    add = mybir.AluOpType.add
    prelu = mybir.ActivationFunctionType.Prelu

    with tile.TileContext(nc) as tc, ExitStack() as ctx:
        cpool = ctx.enter_context(tc.tile_pool(name="cpool", bufs=1))
        ct = cpool.tile([P, 4], f32)
        nc.sync.dma_start(ct[:], c_d.ap())

        xin = ctx.enter_context(tc.tile_pool(name="xin", bufs=4))
        pp = ctx.enter_context(tc.tile_pool(name="pp", bufs=2))
        op = ctx.enter_context(tc.tile_pool(name="op", bufs=3))

        for i in range(n_tiles):
            xt = xin.tile([P, f_tile], bf16)
            nc.gpsimd.dma_start(xt[:], x_d.ap()[:, bass.ts(i, f_tile)])

            pt = pp.tile([P, f_tile], bf16)
            nc.scalar.activation(pt[:], xt[:], prelu,
                                 bias=ct[:, 0:1], alpha=ct[:, 1:2])
            ot = op.tile([P, f_tile], bf16)
            nc.vector.tensor_scalar(ot[:], pt[:], ct[:, 2:3], ct[:, 3:4],
                                    mul, add)

            nc.sync.dma_start(o_d.ap()[:, bass.ts(i, f_tile)], ot[:])

    nc.compile()
    return nc


def _build_bass(T, free=FREE, f_tile=F_TILE):
    """Emit + compile the Bass/Tile program for term count T."""
    from contextlib import ExitStack

    import concourse.bass as bass
    import concourse.tile as tile
    from concourse import bacc, mybir

    nc = bacc.Bacc("TRN2", target_bir_lowering=False, debug=False,
                   num_devices=N_CORES)
    f32 = mybir.dt.float32
    bf16 = mybir.dt.bfloat16
    x_d = nc.dram_tensor("x", [P, free], bf16, kind="ExternalInput")
    c_d = nc.dram_tensor("consts", [P, 2 + 2 * T], f32, kind="ExternalInput")
    o_d = nc.dram_tensor("out", [P, free], bf16, kind="ExternalOutput")
    n_tiles = free // f_tile
    assert n_tiles * f_tile == free

    mul = mybir.AluOpType.mult
    add = mybir.AluOpType.add
    relu = mybir.ActivationFunctionType.Relu

    with tile.TileContext(nc) as tc, ExitStack() as ctx:
        cpool = ctx.enter_context(tc.tile_pool(name="cpool", bufs=1))
        ct = cpool.tile([P, 2 + 2 * T], f32)
        nc.sync.dma_start(ct[:], c_d.ap())

        xin = ctx.enter_context(tc.tile_pool(name="xin", bufs=3))
        fp = ctx.enter_context(tc.tile_pool(name="fp", bufs=2))
        rp = ctx.enter_context(tc.tile_pool(name="rp", bufs=2))
        op = ctx.enter_context(tc.tile_pool(name="op", bufs=3))

        for i in range(n_tiles):
            xt = xin.tile([P, f_tile], bf16)
            # loads issued from the ACT sequencer (qACT HWDGE queue) so that
            # stores (qSP via nc.sync) never head-of-line-block the loads
            nc.scalar.dma_start(xt[:], x_d.ap()[:, bass.ts(i, f_tile)])

            acc = fp.tile([P, f_tile], bf16)
            nc.vector.tensor_scalar(acc[:], xt[:], ct[:, 1:2], ct[:, 0:1], mul, add)

            for j in range(T):
                rt = rp.tile([P, f_tile], bf16)
                nc.scalar.activation(rt[:], xt[:], relu,
                                     bias=ct[:, 2 + 2 * j:3 + 2 * j])
                ot = op.tile([P, f_tile], bf16)
                nc.vector.scalar_tensor_tensor(ot[:], rt[:],
                                               ct[:, 3 + 2 * j:4 + 2 * j],
                                               acc[:], mul, add)
                acc = ot

            nc.sync.dma_start(o_d.ap()[:, bass.ts(i, f_tile)], acc[:])

    nc.compile()
    return nc


_NC_CACHE = {}


def _get_nc(T):
    if T not in _NC_CACHE:
        if T == "prelu_fold":
            _NC_CACHE[T] = _build_bass_prelu_fold()
        elif T == "prelu":
            _NC_CACHE[T] = _build_bass_prelu()
        else:
            _NC_CACHE[T] = _build_bass(T)
    return _NC_CACHE[T]


def _prepare(x, coefficients_vect):
    """Compile (or fetch) the program and build per-core input maps."""
    x = np.asarray(x)
    assert x.shape == (N_BATCH, C, 256, 256)
    cv = np.asarray(coefficients_vect, np.float32)

    alpha, beta, terms, T = _build_pwl(cv)
    T = max(T, 1)
    if _prelu_ok(beta, terms, T) and _prelu_fold_ok(alpha, beta, terms, cv):
        consts = _consts_prelu_fold(alpha, beta, terms)
        nc = _get_nc("prelu_fold")
    elif _prelu_ok(beta, terms, T):
        consts = _consts_prelu(alpha, beta, terms)
        nc = _get_nc("prelu")
    else:
        consts = _consts_array(alpha, beta, terms, T)
        nc = _get_nc(T)
    in_maps = [
        {"x": np.ascontiguousarray(
            x[i * BATCH_PER_CORE:(i + 1) * BATCH_PER_CORE]
        ).reshape(P, FREE).astype(IO_NP),
         "consts": consts}
        for i in range(N_CORES)
    ]
    return nc, in_maps


def kernel(x, coefficients_vect, size):
    assert int(size) == SIZE

    from concourse.bass_utils import run_bass_kernel_spmd

    nc, in_maps = _prepare(x, coefficients_vect)
    res = run_bass_kernel_spmd(nc, in_maps, list(range(N_CORES))).results
    out = np.concatenate(
        [r["out"].astype(np.float32).reshape(BATCH_PER_CORE, C, 256, 256)
         for r in res], axis=0
    )
    return out


# revision 17
# speedup vs baseline: 1.1571x; 1.0061x over previous
"""Trainium2 Bass kernel for nn_DeepBSpline (per-channel uniform-knot linear
B-spline activation with linear extrapolation).

Approach: the whole op (clamp + bin + two gathers + lerp + extrapolation) is,
per channel, a single continuous piecewise-linear function of x with at most
50 pieces whose kinks sit at the (compile-time-known) knot grid.  At call
time the host compresses the coefficient table into its minimal relu basis

    f_c(x) = alpha_c + beta_c * x + sum_j D_cj * relu(x - b_cj)

keeping only kinks with a non-negligible slope change.  The device kernel is
then gather-free: one fused multiply-add (VectorE tensor_scalar) plus, per
kept kink, one biased Relu (ScalarE activation) and one fused multiply-add
(VectorE scalar_tensor_tensor), with per-partition scalars carrying the
per-channel constants.  The number of kept kinks T is the only thing baked
into the program; all values flow through a small "consts" input tensor, so
the compiled NEFF is reusable across coefficient values with the same T.

The op is HBM-bandwidth bound (pure streaming elementwise), so device I/O is
done in bfloat16: the host casts x to bf16, the device computes in bf16
(per-partition scalar constants stay fp32), and the bf16 result is upcast on
the host.  This halves HBM traffic vs fp32.  bf16 adds ~2^-9 relative
rounding per step, orders of magnitude below the 2e-2 gate (the function has
slope <= max per-bin slope, so x-quantization error stays relative).

Sharding: data-parallel over the batch dim — 8 cores x 2 batches each; each
core's (2, 64, 256, 256) slab is viewed as [128 partitions, 65536] with
partition p = b*64 + c, so per-channel constants become per-partition scalars.
"""

import os
import sys

import numpy as np

for _p in ("/opt/trn_rl_repo", "/root/.axon_site", "/root/.axon_site/_ro/trn_rl_repo",
           "/root/.axon_site/_ro/pypackages"):
    if os.path.isdir(_p) and _p not in sys.path:
        sys.path.append(_p)

import ml_dtypes

GRID = 0.16
SIZE = 51
HALF = SIZE // 2
C = 64
N_BATCH = 16
HW = 256 * 256
N_CORES = 8
P = 128                      # partitions = 2 batches x 64 channels
BATCH_PER_CORE = N_BATCH // N_CORES
FREE = BATCH_PER_CORE * C * HW // P   # 65536 free-dim elements per partition
F_TILE = 8192
IO_NP = ml_dtypes.bfloat16


def _build_pwl(coefficients_vect, tol_rel=1e-4):
    """Compress the spline table to relu-basis PWL coefficients (float64).

    Returns alpha[C], beta[C], terms (per channel list of (kink_x, slope_delta)),
    and the max term count across channels.
    """
    cv = np.asarray(coefficients_vect, np.float64).reshape(C, SIZE)
    slopes_x = np.diff(cv, axis=1) / GRID          # (C, 50) per-bin slopes
    dd = np.diff(slopes_x, axis=1)                 # (C, 49) slope changes at knots 1..49
    scale = np.abs(dd).max() + 1e-30
    keep = np.abs(dd) > tol_rel * scale
    alpha = np.empty(C)
    beta = np.empty(C)
    terms = []
    max_terms = 0
    for c in range(C):
        ks = [0] + list(np.nonzero(keep[c])[0] + 1) + [SIZE - 1]
        # refit chords so the PWL interpolates the exact table values at the
        # kept kinks and both endpoints
        k0, k1 = ks[0], ks[1]
        s0 = (cv[c, k1] - cv[c, k0]) / ((k1 - k0) * GRID)
        beta[c] = s0
        alpha[c] = cv[c, k0] - (k0 - HALF) * GRID * s0
        t = []
        prev_s = s0
        for i in range(1, len(ks) - 1):
            ka, kb = ks[i], ks[i + 1]
            s = (cv[c, kb] - cv[c, ka]) / ((kb - ka) * GRID)
            t.append(((ka - HALF) * GRID, s - prev_s))
            prev_s = s
        terms.append(t)
        max_terms = max(max_terms, len(t))
    return alpha, beta, terms, max_terms


def _consts_array(alpha, beta, terms, T):
    """[P, 2+2T] float32: per partition (b*64+c): alpha, beta, (-b_j, D_j)*T."""
    K = 2 + 2 * T
    a = np.zeros((C, K), np.float32)
    a[:, 0] = np.asarray(alpha, np.float32)
    a[:, 1] = np.asarray(beta, np.float32)
    for c in range(C):
        for j, (b, d) in enumerate(terms[c]):
            a[c, 2 + 2 * j] = np.float32(-b)
            a[c, 3 + 2 * j] = np.float32(d)
    return np.tile(a, (P // C, 1)).astype(np.float32)


def _consts_prelu(alpha, beta, terms):
    """[P, 4] float32 per partition: -b, beta1/beta2, beta2, f(b).

    For T==1 the PWL is two lines meeting at b with slopes beta1 = beta and
    beta2 = beta + D, so  f(x) = beta2 * prelu(x - b, beta1/beta2) + f(b).
    """
    a = np.zeros((C, 4), np.float32)
    for c in range(C):
        b, Dv = terms[c][0] if terms[c] else (0.0, 0.0)
        b1 = beta[c]
        b2 = beta[c] + Dv
        a[c, 0] = -b
        a[c, 1] = b1 / b2
        a[c, 2] = b2
        a[c, 3] = alpha[c] + beta[c] * b
    return np.tile(a, (P // C, 1)).astype(np.float32)


def _prelu_ok(beta, terms, T):
    """Prelu path needs T==1 and a non-degenerate right slope everywhere."""
    if T != 1:
        return False
    beta1 = np.asarray(beta)
    beta2 = np.array([beta[c] + (terms[c][0][1] if terms[c] else 0.0)
                      for c in range(C)])
    sc = max(np.abs(beta1).max(), np.abs(beta2).max(), 1e-30)
    return bool(np.abs(beta2).min() > 1e-6 * sc)


def _prelu_fold_ok(alpha, beta, terms, cv):
    """ACT-only path: f = Prelu(beta2*x - beta2*b, beta1/beta2) exactly when
    beta2 > 0 (positive homogeneity) and f(b) ~ 0 (skippable offset)."""
    beta2 = np.array([beta[c] + (terms[c][0][1] if terms[c] else 0.0)
                      for c in range(C)])
    fb = np.array([alpha[c] + beta[c] * (terms[c][0][0] if terms[c] else 0.0)
                   for c in range(C)])
    scale = np.abs(np.asarray(cv)).max() + 1e-30
    return bool(beta2.min() > 0 and np.abs(fb).max() < 1e-5 * scale)


def _consts_prelu_fold(alpha, beta, terms):
    """[P, 3] float32 per partition: -beta2*b (bias), beta2 (scale), beta1/beta2."""
    a = np.zeros((C, 3), np.float32)
    for c in range(C):
        b, Dv = terms[c][0] if terms[c] else (0.0, 0.0)
        b2 = beta[c] + Dv
        a[c, 0] = -b2 * b
        a[c, 1] = b2
        a[c, 2] = beta[c] / b2
    return np.tile(a, (P // C, 1)).astype(np.float32)


# Uniform 4096-elem tiles with one xin buffer per tile: the tile-pool
# flow-control guards are embedded as descriptor-level waits in the DMA
# ring, and any guard stall idles the whole 16-engine pool (a pure-DMA
# probe with guarded loads ran 105us vs the ~92us work-conserving bound).
# With bufs == n_tiles the load ring is completely guard-free.
TILE_SCHED = (4096,) * 15 + (2048, 2048)
assert sum(TILE_SCHED) == FREE


def _build_bass_prelu_fold(free=FREE, sched=TILE_SCHED):
    """ACT-only program: per tile a single ScalarE Prelu, nothing else.

    The op is HBM-bound (per-core DMA engine pool ~25 B/ns x 16 needs
    ~10.4 us/tile for load+store vs 7.3 us of ScalarE Prelu), so the whole
    game is keeping the DMA engines saturated: deep load prefetch (xin
    bufs), load issue on the otherwise-idle GpSimd SWDGE queue, store issue
    on the idle SP queue, and no VectorE stage at all.
    """
    from contextlib import ExitStack

    import concourse.bass as bass
    import concourse.tile as tile
    from concourse import bacc, mybir

    nc = bacc.Bacc("TRN2", target_bir_lowering=False, debug=False,
                   num_devices=N_CORES)
    f32 = mybir.dt.float32
    bf16 = mybir.dt.bfloat16
    x_d = nc.dram_tensor("x", [P, free], bf16, kind="ExternalInput")
    c_d = nc.dram_tensor("consts", [P, 3], f32, kind="ExternalInput")
    o_d = nc.dram_tensor("out", [P, free], bf16, kind="ExternalOutput")
    assert sum(sched) == free

    prelu = mybir.ActivationFunctionType.Prelu

    with tile.TileContext(nc) as tc, ExitStack() as ctx:
        cpool = ctx.enter_context(tc.tile_pool(name="cpool", bufs=1))
        ct = cpool.tile([P, 3], f32)
        nc.gpsimd.dma_start(ct[:], c_d.ap())

        xin = ctx.enter_context(tc.tile_pool(name="xin", bufs=17))
        op = ctx.enter_context(tc.tile_pool(name="op", bufs=8))

        # Loads ride the SP HWDGE queue so they pipeline to the full xin
        # depth (SWDGE flushes between dma_starts, serializing a load
        # stream).  Stores go out on the GpSimd SWDGE queue: they only
        # become ready at ACT rate (~7.3us/tile), so SWDGE's flush-per-dma
        # never throttles them, and keeping them off the load queue avoids
        # head-of-line blocking.
        off = 0
        for sz in sched:
            sl = bass.DynSlice(off, sz)
            off += sz
            xt = xin.tile([P, sz], bf16)
            nc.sync.dma_start(xt[:], x_d.ap()[:, sl])

            ot = op.tile([P, sz], bf16)
            nc.scalar.activation(ot[:], xt[:], prelu,
                                 bias=ct[:, 0:1], scale=ct[:, 1:2],
                                 alpha=ct[:, 2:3])

            nc.gpsimd.dma_start(o_d.ap()[:, sl], ot[:])

    nc.compile()
    return nc


def _build_bass_prelu(free=FREE, f_tile=F_TILE):
    """Prelu-path program: per tile one ScalarE Prelu + one VectorE FMA.

    The op is HBM-bound; DMA issue must never queue behind long compute on
    the same sequencer, so loads go out on the (otherwise idle) PE sequencer
    and stores on the SP sequencer.  ScalarE runs only the Prelu, VectorE
    only the (fast-mode) tensor_scalar.
    """
    from contextlib import ExitStack

    import concourse.bass as bass
    import concourse.tile as tile
    from concourse import bacc, mybir

    nc = bacc.Bacc("TRN2", target_bir_lowering=False, debug=False,
                   num_devices=N_CORES)
    f32 = mybir.dt.float32
    bf16 = mybir.dt.bfloat16
    x_d = nc.dram_tensor("x", [P, free], bf16, kind="ExternalInput")
    c_d = nc.dram_tensor("consts", [P, 4], f32, kind="ExternalInput")
    o_d = nc.dram_tensor("out", [P, free], bf16, kind="ExternalOutput")
    n_tiles = free // f_tile
    assert n_tiles * f_tile == free

    mul = mybir.AluOpType.mult
    add = mybir.AluOpType.add
    prelu = mybir.ActivationFunctionType.Prelu

    with tile.TileContext(nc) as tc, ExitStack() as ctx:
        cpool = ctx.enter_context(tc.tile_pool(name="cpool", bufs=1))
        ct = cpool.tile([P, 4], f32)
        nc.sync.dma_start(ct[:], c_d.ap())

        xin = ctx.enter_context(tc.tile_pool(name="xin", bufs=4))
        pp = ctx.enter_context(tc.tile_pool(name="pp", bufs=2))
        op = ctx.enter_context(tc.tile_pool(name="op", bufs=3))

        for i in range(n_tiles):
            xt = xin.tile([P, f_tile], bf16)
            nc.gpsimd.dma_start(xt[:], x_d.ap()[:, bass.ts(i, f_tile)])

            pt = pp.tile([P, f_tile], bf16)
            nc.scalar.activation(pt[:], xt[:], prelu,
                                 bias=ct[:, 0:1], alpha=ct[:, 1:2])
            ot = op.tile([P, f_tile], bf16)
            nc.vector.tensor_scalar(ot[:], pt[:], ct[:, 2:3], ct[:, 3:4],
                                    mul, add)

            nc.sync.dma_start(o_d.ap()[:, bass.ts(i, f_tile)], ot[:])

    nc.compile()
    return nc


def _build_bass(T, free=FREE, f_tile=F_TILE):
    """Emit + compile the Bass/Tile program for term count T."""
    from contextlib import ExitStack

    import concourse.bass as bass
    import concourse.tile as tile
    from concourse import bacc, mybir

    nc = bacc.Bacc("TRN2", target_bir_lowering=False, debug=False,
                   num_devices=N_CORES)
    f32 = mybir.dt.float32
    bf16 = mybir.dt.bfloat16
    x_d = nc.dram_tensor("x", [P, free], bf16, kind="ExternalInput")
    c_d = nc.dram_tensor("consts", [P, 2 + 2 * T], f32, kind="ExternalInput")
    o_d = nc.dram_tensor("out", [P, free], bf16, kind="ExternalOutput")
    n_tiles = free // f_tile
    assert n_tiles * f_tile == free

    mul = mybir.AluOpType.mult
    add = mybir.AluOpType.add
    relu = mybir.ActivationFunctionType.Relu

    with tile.TileContext(nc) as tc, ExitStack() as ctx:
        cpool = ctx.enter_context(tc.tile_pool(name="cpool", bufs=1))
        ct = cpool.tile([P, 2 + 2 * T], f32)
        nc.sync.dma_start(ct[:], c_d.ap())

        xin = ctx.enter_context(tc.tile_pool(name="xin", bufs=3))
        fp = ctx.enter_context(tc.tile_pool(name="fp", bufs=2))
        rp = ctx.enter_context(tc.tile_pool(name="rp", bufs=2))
        op = ctx.enter_context(tc.tile_pool(name="op", bufs=3))

        for i in range(n_tiles):
            xt = xin.tile([P, f_tile], bf16)
            # loads issued from the ACT sequencer (qACT HWDGE queue) so that
            # stores (qSP via nc.sync) never head-of-line-block the loads
            nc.scalar.dma_start(xt[:], x_d.ap()[:, bass.ts(i, f_tile)])

            acc = fp.tile([P, f_tile], bf16)
            nc.vector.tensor_scalar(acc[:], xt[:], ct[:, 1:2], ct[:, 0:1], mul, add)

            for j in range(T):
                rt = rp.tile([P, f_tile], bf16)
                nc.scalar.activation(rt[:], xt[:], relu,
                                     bias=ct[:, 2 + 2 * j:3 + 2 * j])
                ot = op.tile([P, f_tile], bf16)
                nc.vector.scalar_tensor_tensor(ot[:], rt[:],
                                               ct[:, 3 + 2 * j:4 + 2 * j],
                                               acc[:], mul, add)
                acc = ot

            nc.sync.dma_start(o_d.ap()[:, bass.ts(i, f_tile)], acc[:])

    nc.compile()
    return nc


_NC_CACHE = {}


def _get_nc(T):
    if T not in _NC_CACHE:
        if T == "prelu_fold":
            _NC_CACHE[T] = _build_bass_prelu_fold()
        elif T == "prelu":
            _NC_CACHE[T] = _build_bass_prelu()
        else:
            _NC_CACHE[T] = _build_bass(T)
    return _NC_CACHE[T]


def _prepare(x, coefficients_vect):
    """Compile (or fetch) the program and build per-core input maps."""
    x = np.asarray(x)
    assert x.shape == (N_BATCH, C, 256, 256)
    cv = np.asarray(coefficients_vect, np.float32)

    alpha, beta, terms, T = _build_pwl(cv)
    T = max(T, 1)
    if _prelu_ok(beta, terms, T) and _prelu_fold_ok(alpha, beta, terms, cv):
        consts = _consts_prelu_fold(alpha, beta, terms)
        nc = _get_nc("prelu_fold")
    elif _prelu_ok(beta, terms, T):
        consts = _consts_prelu(alpha, beta, terms)
        nc = _get_nc("prelu")
    else:
        consts = _consts_array(alpha, beta, terms, T)
        nc = _get_nc(T)
    in_maps = [
        {"x": np.ascontiguousarray(
            x[i * BATCH_PER_CORE:(i + 1) * BATCH_PER_CORE]
        ).reshape(P, FREE).astype(IO_NP),
         "consts": consts}
        for i in range(N_CORES)
    ]
    return nc, in_maps


def kernel(x, coefficients_vect, size):
    assert int(size) == SIZE

    from concourse.bass_utils import run_bass_kernel_spmd

    nc, in_maps = _prepare(x, coefficients_vect)
    res = run_bass_kernel_spmd(nc, in_maps, list(range(N_CORES))).results
    out = np.concatenate(
        [r["out"].astype(np.float32).reshape(BATCH_PER_CORE, C, 256, 256)
         for r in res], axis=0
    )
    return out
